# revision 18
# baseline (speedup 1.0000x reference)
"""AxialAttention (relative-position skew attention) Trainium2 Bass kernel.

Sharding: data-parallel over batch B=8 across 8 NeuronCores (one batch
element per core, SPMD). Each core computes its full attention locally;
no collectives.

Per-core algorithm (N=1024, C=512, H=8, d=64):
  qT_h = 0.125 * (Wq_h^T @ x^T)        [64, N]  (scale folded into q)
  kT_h = Wk_h^T @ x^T                  [64, N]
  V    = x @ Wv                        [N, C]
  S    = q_h @ k_h^T                   (PE, bf16)
  G    = q_h @ (8*Er_q)^T + k_h @ Er_k^T
  skew(G) via DRAM zero-stuffed buffer trick:
     write G rows into Z viewed [N, N+1] cols 1..N (col 0 = zeros);
     the skewed matrix is exactly Z.flat[N : N+N*N] viewed [N, N].
  P = exp(S + skew(G)); rowsum via ACT accum_out; normalize (DVE)
  write P rows into another stuffed buffer Za;
  sv^T + VEr^T accumulated in one PSUM group:
     sv^T  = sum_mt  V[mt, h]^T-as-lhsT @ (P^T tile via DMA-xbar of Za plain view)
     VEr^T = sum_mt  Er_v[mt]-as-lhsT  @ (skew(P)^T tile via DMA-xbar of Za skew view)
  out = concat_heads(o)^T-as-lhsT @ Wo + bo
"""

import numpy as np
import ml_dtypes

import concourse.bass as bass
import concourse.mybir as mybir
import concourse.tile as tile
from concourse.bass_utils import run_bass_kernel_spmd
from contextlib import ExitStack

BF = mybir.dt.bfloat16
F8 = mybir.dt.float8e4
F32 = mybir.dt.float32
P = 128
N = 1024
C = 512
D = 64
H = 8
NT = N // P   # 8 strips/tiles
CT = C // P   # 4 c-tiles
CHW = 512     # matmul N-chunk (one PSUM bank of fp32)
CH = N // CHW # 2 chunks

_CACHE = {}


def _install_lane_partitioning():
    """Split HWDGE DMAs across both physical rings (qSPDynamicHW via
    nc.sync, qActDynamicHW via nc.scalar) for parallel transfer streams.
    Tile's DMA-completion sems assume per-lane FIFO completion, which
    breaks when two rings share sem lanes — so partition the 8 lanes by
    issuing engine: SP -> lanes 0-3, ACT -> lanes 4-7."""
    import concourse.tile_sem_assignment as tsa
    from concourse.tile_scheduler import DMAInst
    if getattr(tsa.TileClockTick, "_lane_partitioned", False):
        return
    orig = tsa.TileClockTick._assign_tick

    def patched(self, inst):
        if isinstance(inst, DMAInst) and inst.engine != mybir.EngineType.Pool:
            if inst.engine == mybir.EngineType.Activation:
                i = getattr(self, "_act_lane", 0)
                self.next_hw_dma_idx = 4 + (i % 4)
                self._act_lane = i + 1
            else:
                i = getattr(self, "_sp_lane", 0)
                self.next_hw_dma_idx = i % 4
                self._sp_lane = i + 1
        return orig(self, inst)

    tsa.TileClockTick._assign_tick = patched
    tsa.TileClockTick._lane_partitioned = True


def _split_multi_waits(nc):
    """Walrus in this toolchain allows only ONE sync-wait per instruction
    (setupSyncWait: 'Too many sync wait commands'). Tile emits multi-wait
    instructions (esp. the kernel-tail Drain). Split extras into preceding
    single-wait InstEventSemaphore ops on the same engine queue."""
    n = 0
    for bb in nc.m.functions[0].blocks:
        insts = list(bb.instructions)
        new = []
        for inst in insts:
            si = inst.sync_info
            waits = list(si.on_wait) if si is not None and si.on_wait else []
            if len(waits) > 1:
                for w in waits[:-1]:
                    n += 1
                    new.append(mybir.InstEventSemaphore(
                        name=f"I-wsplit-{n}",
                        engine=inst.engine, ins=[], outs=[],
                        sync_info=mybir.SyncInfo(on_wait=[w], on_update=[])))
                si.on_wait = waits[-1:]
            new.append(inst)
        bb.instructions = new
    return n


def _build():
    _install_lane_partitioning()
    nc = bass.Bass()
    xT = nc.declare_dram_parameter("xT", [C, N], BF, isOutput=False)
    wq = nc.declare_dram_parameter("wq", [C, C], BF, isOutput=False)
    wk = nc.declare_dram_parameter("wk", [C, C], BF, isOutput=False)
    wv = nc.declare_dram_parameter("wv", [C, C], BF, isOutput=False)
    wo = nc.declare_dram_parameter("wo", [C, C], BF, isOutput=False)
    eq8T = nc.declare_dram_parameter("eq8T", [D, N], BF, isOutput=False)
    ekT = nc.declare_dram_parameter("ekT", [D, N], BF, isOutput=False)
    erv = nc.declare_dram_parameter("erv", [N, D], BF, isOutput=False)
    biasb = nc.declare_dram_parameter("biasb", [P, C], F32, isOutput=False)
    ident = nc.declare_dram_parameter("ident", [P, P], F8, isOutput=False)
    out = nc.declare_dram_parameter("out", [N, C], F32, isOutput=True)

    Exp = mybir.ActivationFunctionType.Exp
    ZSZ = N * (N + 1)

    with ExitStack() as ctx:
        tc = ctx.enter_context(tile.TileContext(nc))
        const = ctx.enter_context(tc.tile_pool(name="const", bufs=1))
        work = ctx.enter_context(tc.tile_pool(name="work", bufs=3))
        gpool = ctx.enter_context(tc.tile_pool(name="gev", bufs=3))
        apool = ctx.enter_context(tc.tile_pool(name="attn", bufs=2))
        tpool = ctx.enter_context(tc.tile_pool(name="trp", bufs=1))
        mm_ps = ctx.enter_context(tc.tile_pool(name="mmps", bufs=3, space="PSUM"))
        o_ps = ctx.enter_context(tc.tile_pool(name="ops", bufs=1, space="PSUM"))
        dram = ctx.enter_context(tc.tile_pool(name="dram", bufs=1, space="DRAM"))

        # ---- load constants ----
        xT_sb = []
        for i in range(CT):
            t = const.tile([P, N], BF, name=f"xT{i}")
            nc.sync.dma_start(out=t[:], in_=xT[i * P:(i + 1) * P, :])
            xT_sb.append(t)

        def load_w(param, nm):
            ts = []
            for i in range(CT):
                t = const.tile([P, C], BF, name=f"{nm}{i}")
                nc.sync.dma_start(out=t[:], in_=param[i * P:(i + 1) * P, :])
                ts.append(t)
            return ts

        wq_sb = load_w(wq, "wq")
        wk_sb = load_w(wk, "wk")
        wv_sb = load_w(wv, "wv")
        wo_sb = load_w(wo, "wo")

        ee_sb = const.tile([P, N], BF, name="ee")
        nc.sync.dma_start(out=ee_sb[0:D, :], in_=eq8T[:, :])
        nc.sync.dma_start(out=ee_sb[D:2 * D, :], in_=ekT[:, :])
        erv_sb = []
        for mt in range(NT):
            t = const.tile([P, D], BF, name=f"erv{mt}")
            nc.sync.dma_start(out=t[:], in_=erv[mt * P:(mt + 1) * P, :])
            erv_sb.append(t)
        bias_sb = const.tile([P, C], F32, name="bias")
        nc.sync.dma_start(out=bias_sb[:], in_=biasb[:, :])
        id_sb = const.tile([P, P], F8, name="ident")
        nc.sync.dma_start(out=id_sb[:], in_=ident[:, :])
        # ---- DRAM stuffed scratch buffers (ping-pong across heads) ----
        # The stuffed zero column is written as part of each row-strip
        # write (tiles carry a zeroed col 0), so writes stay contiguous.
        zgs, zas = [], []
        for i in range(2):
            zgs.append(dram.tile([ZSZ], F8, name=f"zg{i}"))
            zas.append(dram.tile([ZSZ], BF, name=f"za{i}"))

        # ---- projections ----
        # qkT_h: [q_h^T (rows 0:64) ; k_h^T (rows 64:128)] for the stacked
        # G matmul (K=128). kT_h: k_h^T at partition base 0 for S's rhs.
        qkT_h = [const.tile([P, N], BF, name=f"qkT{h}") for h in range(H)]
        kT_h = [const.tile([D, N], BF, name=f"kT{h}") for h in range(H)]

        def project(w_sb, scale, place):
            for ct in range(CT):
                pair = work.tile([P, N], BF, tag="pair", name=f"pair{ct}")
                for ch in range(CH):
                    ps = mm_ps.tile([P, CHW], F32, tag="mm", name="psp")
                    for kt in range(CT):
                        nc.tensor.matmul(
                            ps[:],
                            w_sb[kt][:, ct * P:(ct + 1) * P],
                            xT_sb[kt][:, ch * CHW:(ch + 1) * CHW],
                            start=(kt == 0),
                            stop=(kt == CT - 1),
                        )
                    nc.scalar.mul(pair[:, ch * CHW:(ch + 1) * CHW], ps[:], scale)
                place(ct, pair)

        def place_q(ct, pair):
            nc.sync.dma_start(out=qkT_h[2 * ct][0:D, :], in_=pair[0:D, :])
            nc.sync.dma_start(out=qkT_h[2 * ct + 1][0:D, :], in_=pair[D:2 * D, :])

        def place_k(ct, pair):
            nc.sync.dma_start(out=qkT_h[2 * ct][D:2 * D, :], in_=pair[0:D, :])
            nc.sync.dma_start(out=qkT_h[2 * ct + 1][D:2 * D, :], in_=pair[D:2 * D, :])
            nc.sync.dma_start(out=kT_h[2 * ct][0:D, :], in_=pair[0:D, :])
            nc.sync.dma_start(out=kT_h[2 * ct + 1][0:D, :], in_=pair[D:2 * D, :])

        project(wq_sb, 0.125, place_q)
        project(wk_sb, 1.0, place_k)

        vfull = []
        for mt in range(NT):
            ps = mm_ps.tile([P, C], F32, tag="mm", name="psv")
            for kt in range(CT):
                nc.tensor.matmul(
                    ps[:],
                    xT_sb[kt][:, mt * P:(mt + 1) * P],
                    wv_sb[kt][:],
                    start=(kt == 0),
                    stop=(kt == CT - 1),
                )
            t = const.tile([P, C], BF, name=f"v{mt}")
            nc.scalar.copy(t[:], ps[:])
            vfull.append(t)

        oT_all = [const.tile([P, N], BF, name=f"oT{ct}") for ct in range(CT)]

        # ---- per-head attention (software-pipelined) ----
        # A(h): G strips -> DRAM; B(h): S + skew-add + softmax -> DRAM;
        # C(h): xbar-transposed reads + sv/VEr matmuls. C(h) is interleaved
        # with A(h+1) so PE always has independent matmuls to chew on while
        # the xbar reads stream in. Plain DMAs ride SWDGE (gpsimd) to keep
        # the SP HWDGE ring exclusively for the xbar transposes.
        ROWB = P * (N + 1)

        def views(h):
            zg, za = zgs[h % 2], zas[h % 2]
            return (zg, za,
                    za[:].rearrange("(n m) -> n m", m=N + 1),
                    zg[N:N + N * N].rearrange("(n m) -> n m", m=N),
                    za[N:N + N * N].rearrange("(n m) -> n m", m=N))

        def phase_a_strip(h, s):
            zg = zgs[h % 2]
            gp = mm_ps.tile([P, N], F32, tag="mm", name="gp")
            for ch in range(CH):
                sl = slice(ch * CHW, (ch + 1) * CHW)
                nc.tensor.matmul(
                    gp[:, sl], qkT_h[h][:, s * P:(s + 1) * P], ee_sb[:, sl],
                    start=True, stop=True, skip_group_check=True)
            gsb = gpool.tile([P, N + 1], F8, tag="gsb", name="gsb")
            nc.vector.memset(gsb[:, 0:1], 0.0)
            nc.vector.tensor_copy(gsb[:, 1:N + 1], gp[:])
            dst = zg[s * ROWB:(s + 1) * ROWB].rearrange("(p m) -> p m", m=N + 1)
            nc.gpsimd.dma_start(out=dst[:, :], in_=gsb[:])

        def skg_prefetch(h):
            _, _, _, zg_skew, _ = views(h)
            skgs = []
            for s in range(NT):
                skg = apool.tile([P, N], F8, tag=f"skg{s}", bufs=1,
                                 name=f"skg{s}")
                nc.sync.dma_start(out=skg[:], in_=zg_skew[s * P:(s + 1) * P, :])
                skgs.append(skg)
            return skgs

        def phase_b_strip(h, s, skgs):
            za = zas[h % 2]
            sp = mm_ps.tile([P, N], F32, tag="mm", name="sp")
            for ch in range(CH):
                sl = slice(ch * CHW, (ch + 1) * CHW)
                nc.tensor.matmul(
                    sp[:, sl], qkT_h[h][0:D, s * P:(s + 1) * P],
                    kT_h[h][0:D, sl],
                    start=True, stop=False, skip_group_check=True)
                nc.tensor.matmul(
                    sp[:, sl], id_sb[:], skgs[s][:, sl],
                    start=False, stop=True, skip_group_check=True)
            aexp = apool.tile([P, N], BF, tag="aexp", name="aexp")
            rs = work.tile([P, 1], F32, tag="rs", name="rs")
            nc.scalar.activation(aexp[:], sp[:], Exp, accum_out=rs[:])
            rinv = work.tile([P, 1], F32, tag="rinv", name="rinv")
            nc.vector.reciprocal(rinv[:], rs[:])
            anorm = apool.tile([P, N + 1], BF, tag="anorm", name="anorm")
            nc.vector.memset(anorm[:, 0:1], 0.0)
            nc.vector.tensor_scalar_mul(anorm[:, 1:N + 1], aexp[:], rinv[:])
            dst = za[s * ROWB:(s + 1) * ROWB].rearrange("(p m) -> p m", m=N + 1)
            nc.gpsimd.dma_start(out=dst[:, :], in_=anorm[:])

        def xbar_prefetch(h):
            _, _, za_rows, _, za_skew = views(h)
            ats, sats = [], []
            for mt in range(NT):
                at = tpool.tile([P, N], BF, tag=f"at{mt}", bufs=1,
                                name=f"at{mt}")
                nc.sync.dma_start(
                    out=at[:], in_=za_rows[:, 1 + mt * P:1 + (mt + 1) * P],
                    transpose=True)
                sat = tpool.tile([P, N], BF, tag=f"sat{mt}", bufs=1,
                                 name=f"sat{mt}")
                nc.sync.dma_start(
                    out=sat[:], in_=za_skew[:, mt * P:(mt + 1) * P],
                    transpose=True)
                ats.append(at)
                sats.append(sat)
            return ats, sats

        def phase_c_mt(h, mt, op, ats, sats):
            for ch in range(CH):
                sl = slice(ch * CHW, (ch + 1) * CHW)
                nc.tensor.matmul(
                    op[:, sl], vfull[mt][:, h * D:(h + 1) * D],
                    ats[mt][:, sl],
                    start=(mt == 0), stop=False, skip_group_check=True)
                nc.tensor.matmul(
                    op[:, sl], erv_sb[mt][:], sats[mt][:, sl],
                    start=False, stop=(mt == NT - 1),
                    skip_group_check=True)

        def otile_evict(h, op):
            ot = work.tile([D, N], BF, tag="ot", name="ot")
            nc.scalar.copy(ot[:], op[:])
            nc.sync.dma_start(
                out=oT_all[h // 2][(h % 2) * D:(h % 2 + 1) * D, :], in_=ot[:])

        # Steady state: C(h) || B(h+1) || A(h+2), one strip of each per step.
        for s in range(NT):
            phase_a_strip(0, s)
        skgs = skg_prefetch(0)
        for s in range(NT):
            phase_b_strip(0, s, skgs)
            phase_a_strip(1, s)
        for h in range(H):
            ats, sats = xbar_prefetch(h)
            if h + 1 < H:
                skgs = skg_prefetch(h + 1)
            op = o_ps.tile([D, N], F32, tag="o", name="op")
            for i in range(NT):
                phase_c_mt(h, i, op, ats, sats)
                if h + 1 < H:
                    phase_b_strip(h + 1, i, skgs)
                if h + 2 < H:
                    phase_a_strip(h + 2, i)
            otile_evict(h, op)

        # ---- output projection ----
        for s in range(NT):
            fp = mm_ps.tile([P, C], F32, tag="mm", name="fp")
            for ct in range(CT):
                nc.tensor.matmul(
                    fp[:], oT_all[ct][:, s * P:(s + 1) * P], wo_sb[ct][:],
                    start=(ct == 0), stop=(ct == CT - 1))
            fo = work.tile([P, C], F32, tag="fo", name="fo")
            nc.vector.tensor_add(fo[:], fp[:], bias_sb[:])
            nc.sync.dma_start(out=out[s * P:(s + 1) * P, :], in_=fo[:])

    _split_multi_waits(nc)
    return nc


def kernel(x, Wq, Wk, Wv, Er_q, Er_k, Er_v, Wo, bo, **kw):
    if "nc" not in _CACHE:
        _CACHE["nc"] = _build()
    nc = _CACHE["nc"]
    bf = ml_dtypes.bfloat16
    x = np.asarray(x, dtype=np.float32)
    base = {
        "wq": np.asarray(Wq, np.float32).astype(bf),
        "wk": np.asarray(Wk, np.float32).astype(bf),
        "wv": np.asarray(Wv, np.float32).astype(bf),
        "wo": np.asarray(Wo, np.float32).astype(bf),
        "eq8T": np.ascontiguousarray(np.asarray(Er_q, np.float32).T * 8.0).astype(bf),
        "ekT": np.ascontiguousarray(np.asarray(Er_k, np.float32).T).astype(bf),
        "erv": np.asarray(Er_v, np.float32).astype(bf),
        "biasb": np.ascontiguousarray(
            np.broadcast_to(np.asarray(bo, np.float32), (P, C))),
        "ident": np.eye(P, dtype=ml_dtypes.float8_e4m3),
    }
    in_maps = []
    for b in range(x.shape[0]):
        m = dict(base)
        m["xT"] = np.ascontiguousarray(x[b].T).astype(bf)
        in_maps.append(m)
    _CACHE["in_maps"] = in_maps
    res = run_bass_kernel_spmd(nc, in_maps, list(range(len(in_maps))))
    return np.stack(
        [np.asarray(res.results[i]["out"], dtype=np.float32)
         for i in range(len(in_maps))])


# revision 19
# speedup vs baseline: 1.4284x; 1.4284x over previous
"""AxialAttention (relative-position skew attention) Trainium2 Bass kernel.

Sharding: data-parallel over batch B=8 across 8 NeuronCores (one batch
element per core, SPMD). Each core computes its full attention locally;
no collectives.

Per-core algorithm (N=1024, C=512, H=8, d=64):
  qT_h = 0.125 * (Wq_h^T @ x^T)        [64, N]  (scale folded into q)
  kT_h = Wk_h^T @ x^T                  [64, N]
  V    = x @ Wv                        [N, C]
  S    = q_h @ k_h^T                   (PE, bf16)
  G    = q_h @ (8*Er_q)^T + k_h @ Er_k^T
  skew(G) via DRAM zero-stuffed buffer trick:
     write G rows into Z viewed [N, N+1] cols 1..N (col 0 = zeros);
     the skewed matrix is exactly Z.flat[N : N+N*N] viewed [N, N].
  P = exp(S + skew(G)); rowsum via ACT accum_out; normalize (DVE)
  write P rows into another stuffed buffer Za;
  sv^T + VEr^T accumulated in one PSUM group:
     sv^T  = sum_mt  V[mt, h]^T-as-lhsT @ (P^T tile via DMA-xbar of Za plain view)
     VEr^T = sum_mt  Er_v[mt]-as-lhsT  @ (skew(P)^T tile via DMA-xbar of Za skew view)
  out = concat_heads(o)^T-as-lhsT @ Wo + bo
"""

import numpy as np
import ml_dtypes

import concourse.bass as bass
import concourse.mybir as mybir
import concourse.tile as tile
from concourse.bass_utils import run_bass_kernel_spmd
from contextlib import ExitStack

BF = mybir.dt.bfloat16
F8 = mybir.dt.float8e4
F32 = mybir.dt.float32
P = 128
N = 1024
C = 512
D = 64
H = 8
NT = N // P   # 8 strips/tiles
CT = C // P   # 4 c-tiles
CHW = 512     # matmul N-chunk (one PSUM bank of fp32)
CH = N // CHW # 2 chunks

_CACHE = {}


def _install_lane_partitioning():
    """Split HWDGE DMAs across both physical rings (qSPDynamicHW via
    nc.sync, qActDynamicHW via nc.scalar) for parallel transfer streams.
    Tile's DMA-completion sems assume per-lane FIFO completion, which
    breaks when two rings share sem lanes — so partition the 8 lanes by
    issuing engine: SP -> lanes 0-3, ACT -> lanes 4-7."""
    import concourse.tile_sem_assignment as tsa
    from concourse.tile_scheduler import DMAInst
    if getattr(tsa.TileClockTick, "_lane_partitioned", False):
        return
    orig = tsa.TileClockTick._assign_tick

    def patched(self, inst):
        if isinstance(inst, DMAInst) and inst.engine != mybir.EngineType.Pool:
            if inst.engine == mybir.EngineType.Activation:
                i = getattr(self, "_act_lane", 0)
                self.next_hw_dma_idx = 4 + (i % 4)
                self._act_lane = i + 1
            else:
                i = getattr(self, "_sp_lane", 0)
                self.next_hw_dma_idx = i % 4
                self._sp_lane = i + 1
        return orig(self, inst)

    tsa.TileClockTick._assign_tick = patched
    tsa.TileClockTick._lane_partitioned = True


def _split_multi_waits(nc):
    """Walrus in this toolchain allows only ONE sync-wait per instruction
    (setupSyncWait: 'Too many sync wait commands'). Tile emits multi-wait
    instructions (esp. the kernel-tail Drain). Split extras into preceding
    single-wait InstEventSemaphore ops on the same engine queue."""
    n = 0
    for bb in nc.m.functions[0].blocks:
        insts = list(bb.instructions)
        new = []
        for inst in insts:
            si = inst.sync_info
            waits = list(si.on_wait) if si is not None and si.on_wait else []
            if len(waits) > 1:
                for w in waits[:-1]:
                    n += 1
                    new.append(mybir.InstEventSemaphore(
                        name=f"I-wsplit-{n}",
                        engine=inst.engine, ins=[], outs=[],
                        sync_info=mybir.SyncInfo(on_wait=[w], on_update=[])))
                si.on_wait = waits[-1:]
            new.append(inst)
        bb.instructions = new
    return n


def _build():
    _install_lane_partitioning()
    nc = bass.Bass()
    xT = nc.declare_dram_parameter("xT", [C, N], BF, isOutput=False)
    wq = nc.declare_dram_parameter("wq", [C, C], BF, isOutput=False)
    wk = nc.declare_dram_parameter("wk", [C, C], BF, isOutput=False)
    wv = nc.declare_dram_parameter("wv", [C, C], BF, isOutput=False)
    wo = nc.declare_dram_parameter("wo", [C, C], BF, isOutput=False)
    eq8T = nc.declare_dram_parameter("eq8T", [D, N], BF, isOutput=False)
    ekT = nc.declare_dram_parameter("ekT", [D, N], BF, isOutput=False)
    erv = nc.declare_dram_parameter("erv", [N, D], BF, isOutput=False)
    biasb = nc.declare_dram_parameter("biasb", [P, C], F32, isOutput=False)
    ident = nc.declare_dram_parameter("ident", [P, P], F8, isOutput=False)
    out = nc.declare_dram_parameter("out", [N, C], F32, isOutput=True)

    Exp = mybir.ActivationFunctionType.Exp
    ZSZ = N * (N + 1)

    with ExitStack() as ctx:
        tc = ctx.enter_context(tile.TileContext(nc))
        const = ctx.enter_context(tc.tile_pool(name="const", bufs=1))
        work = ctx.enter_context(tc.tile_pool(name="work", bufs=3))
        gpool = ctx.enter_context(tc.tile_pool(name="gev", bufs=3))
        apool = ctx.enter_context(tc.tile_pool(name="attn", bufs=2))
        tpool = ctx.enter_context(tc.tile_pool(name="trp", bufs=1))
        mm_ps = ctx.enter_context(tc.tile_pool(name="mmps", bufs=3, space="PSUM"))
        o_ps = ctx.enter_context(tc.tile_pool(name="ops", bufs=1, space="PSUM"))
        dram = ctx.enter_context(tc.tile_pool(name="dram", bufs=1, space="DRAM"))

        # ---- load constants ----
        xT_sb = []
        for i in range(CT):
            t = const.tile([P, N], BF, name=f"xT{i}")
            nc.sync.dma_start(out=t[:], in_=xT[i * P:(i + 1) * P, :])
            xT_sb.append(t)

        def load_w(param, nm):
            ts = []
            for i in range(CT):
                t = const.tile([P, C], BF, name=f"{nm}{i}")
                nc.sync.dma_start(out=t[:], in_=param[i * P:(i + 1) * P, :])
                ts.append(t)
            return ts

        wq_sb = load_w(wq, "wq")
        wk_sb = load_w(wk, "wk")
        wv_sb = load_w(wv, "wv")
        wo_sb = load_w(wo, "wo")

        ee_sb = const.tile([P, N], BF, name="ee")
        nc.sync.dma_start(out=ee_sb[0:D, :], in_=eq8T[:, :])
        nc.sync.dma_start(out=ee_sb[D:2 * D, :], in_=ekT[:, :])
        erv_sb = []
        for mt in range(NT):
            t = const.tile([P, D], BF, name=f"erv{mt}")
            nc.sync.dma_start(out=t[:], in_=erv[mt * P:(mt + 1) * P, :])
            erv_sb.append(t)
        bias_sb = const.tile([P, C], F32, name="bias")
        nc.sync.dma_start(out=bias_sb[:], in_=biasb[:, :])
        id_sb = const.tile([P, P], F8, name="ident")
        nc.sync.dma_start(out=id_sb[:], in_=ident[:, :])
        # ---- DRAM stuffed scratch buffers (ping-pong across heads) ----
        # The stuffed zero column is written as part of each row-strip
        # write (tiles carry a zeroed col 0), so writes stay contiguous.
        zgs, zas = [], []
        for i in range(2):
            zgs.append(dram.tile([ZSZ], F8, name=f"zg{i}"))
            zas.append(dram.tile([ZSZ], BF, name=f"za{i}"))

        # ---- projections ----
        # qkT_h: [q_h^T (rows 0:64) ; k_h^T (rows 64:128)] for the stacked
        # G matmul (K=128). kT_h: k_h^T at partition base 0 for S's rhs.
        qkT_h = [const.tile([P, N], BF, name=f"qkT{h}") for h in range(H)]
        kT_h = [const.tile([D, N], BF, name=f"kT{h}") for h in range(H)]

        def project(w_sb, scale, place):
            for ct in range(CT):
                pair = work.tile([P, N], BF, tag="pair", name=f"pair{ct}")
                for ch in range(CH):
                    ps = mm_ps.tile([P, CHW], F32, tag="mm", name="psp")
                    for kt in range(CT):
                        nc.tensor.matmul(
                            ps[:],
                            w_sb[kt][:, ct * P:(ct + 1) * P],
                            xT_sb[kt][:, ch * CHW:(ch + 1) * CHW],
                            start=(kt == 0),
                            stop=(kt == CT - 1),
                        )
                    nc.scalar.mul(pair[:, ch * CHW:(ch + 1) * CHW], ps[:], scale)
                place(ct, pair)

        def place_q(ct, pair):
            nc.sync.dma_start(out=qkT_h[2 * ct][0:D, :], in_=pair[0:D, :])
            nc.sync.dma_start(out=qkT_h[2 * ct + 1][0:D, :], in_=pair[D:2 * D, :])

        def place_k(ct, pair):
            nc.sync.dma_start(out=qkT_h[2 * ct][D:2 * D, :], in_=pair[0:D, :])
            nc.sync.dma_start(out=qkT_h[2 * ct + 1][D:2 * D, :], in_=pair[D:2 * D, :])
            nc.sync.dma_start(out=kT_h[2 * ct][0:D, :], in_=pair[0:D, :])
            nc.sync.dma_start(out=kT_h[2 * ct + 1][0:D, :], in_=pair[D:2 * D, :])

        project(wq_sb, 0.125, place_q)
        project(wk_sb, 1.0, place_k)

        vfull = []
        for mt in range(NT):
            ps = mm_ps.tile([P, C], F32, tag="mm", name="psv")
            for kt in range(CT):
                nc.tensor.matmul(
                    ps[:],
                    xT_sb[kt][:, mt * P:(mt + 1) * P],
                    wv_sb[kt][:],
                    start=(kt == 0),
                    stop=(kt == CT - 1),
                )
            t = const.tile([P, C], BF, name=f"v{mt}")
            nc.scalar.copy(t[:], ps[:])
            vfull.append(t)

        oT_all = [const.tile([P, N], BF, name=f"oT{ct}") for ct in range(CT)]

        # ---- per-head attention (software-pipelined) ----
        # A(h): G strips -> DRAM; B(h): S + skew-add + softmax -> DRAM;
        # C(h): xbar-transposed reads + sv/VEr matmuls. C(h) is interleaved
        # with A(h+1) so PE always has independent matmuls to chew on while
        # the xbar reads stream in. Plain DMAs ride SWDGE (gpsimd) to keep
        # the SP HWDGE ring exclusively for the xbar transposes.
        ROWB = P * (N + 1)

        def views(h):
            zg, za = zgs[h % 2], zas[h % 2]
            return (zg, za,
                    za[:].rearrange("(n m) -> n m", m=N + 1),
                    zg[N:N + N * N].rearrange("(n m) -> n m", m=N),
                    za[N:N + N * N].rearrange("(n m) -> n m", m=N))

        def phase_a_strip(h, s):
            zg = zgs[h % 2]
            gp = mm_ps.tile([P, N], F32, tag="mm", name="gp")
            for ch in range(CH):
                sl = slice(ch * CHW, (ch + 1) * CHW)
                nc.tensor.matmul(
                    gp[:, sl], qkT_h[h][:, s * P:(s + 1) * P], ee_sb[:, sl],
                    start=True, stop=True, skip_group_check=True)
            gsb = gpool.tile([P, N + 1], F8, tag="gsb", name="gsb")
            nc.vector.memset(gsb[:, 0:1], 0.0)
            nc.vector.tensor_copy(gsb[:, 1:N + 1], gp[:])
            dst = zg[s * ROWB:(s + 1) * ROWB].rearrange("(p m) -> p m", m=N + 1)
            nc.sync.dma_start(out=dst[:, :], in_=gsb[:])

        def skg_prefetch(h):
            _, _, _, zg_skew, _ = views(h)
            skgs = []
            for s in range(NT):
                skg = apool.tile([P, N], F8, tag=f"skg{s}", bufs=1,
                                 name=f"skg{s}")
                nc.sync.dma_start(out=skg[:], in_=zg_skew[s * P:(s + 1) * P, :])
                skgs.append(skg)
            return skgs

        def phase_b_strip(h, s, skgs):
            za = zas[h % 2]
            sp = mm_ps.tile([P, N], F32, tag="mm", name="sp")
            for ch in range(CH):
                sl = slice(ch * CHW, (ch + 1) * CHW)
                nc.tensor.matmul(
                    sp[:, sl], qkT_h[h][0:D, s * P:(s + 1) * P],
                    kT_h[h][0:D, sl],
                    start=True, stop=False, skip_group_check=True)
                nc.tensor.matmul(
                    sp[:, sl], id_sb[:], skgs[s][:, sl],
                    start=False, stop=True, skip_group_check=True)
            aexp = apool.tile([P, N], BF, tag="aexp", name="aexp")
            rs = work.tile([P, 1], F32, tag="rs", name="rs")
            nc.scalar.activation(aexp[:], sp[:], Exp, accum_out=rs[:])
            rinv = work.tile([P, 1], F32, tag="rinv", name="rinv")
            nc.vector.reciprocal(rinv[:], rs[:])
            anorm = apool.tile([P, N + 1], BF, tag="anorm", name="anorm")
            nc.vector.memset(anorm[:, 0:1], 0.0)
            nc.vector.tensor_scalar_mul(anorm[:, 1:N + 1], aexp[:], rinv[:])
            dst = za[s * ROWB:(s + 1) * ROWB].rearrange("(p m) -> p m", m=N + 1)
            nc.sync.dma_start(out=dst[:, :], in_=anorm[:])

        def xbar_prefetch(h):
            _, _, za_rows, _, za_skew = views(h)
            ats, sats = [], []
            for mt in range(NT):
                at = tpool.tile([P, N], BF, tag=f"at{mt}", bufs=1,
                                name=f"at{mt}")
                nc.sync.dma_start(
                    out=at[:], in_=za_rows[:, 1 + mt * P:1 + (mt + 1) * P],
                    transpose=True)
                sat = tpool.tile([P, N], BF, tag=f"sat{mt}", bufs=1,
                                 name=f"sat{mt}")
                nc.sync.dma_start(
                    out=sat[:], in_=za_skew[:, mt * P:(mt + 1) * P],
                    transpose=True)
                ats.append(at)
                sats.append(sat)
            return ats, sats

        def phase_c_mt(h, mt, op, ats, sats):
            for ch in range(CH):
                sl = slice(ch * CHW, (ch + 1) * CHW)
                nc.tensor.matmul(
                    op[:, sl], vfull[mt][:, h * D:(h + 1) * D],
                    ats[mt][:, sl],
                    start=(mt == 0), stop=False, skip_group_check=True)
                nc.tensor.matmul(
                    op[:, sl], erv_sb[mt][:], sats[mt][:, sl],
                    start=False, stop=(mt == NT - 1),
                    skip_group_check=True)

        def otile_evict(h, op):
            ot = work.tile([D, N], BF, tag="ot", name="ot")
            nc.scalar.copy(ot[:], op[:])
            nc.sync.dma_start(
                out=oT_all[h // 2][(h % 2) * D:(h % 2 + 1) * D, :], in_=ot[:])

        # Steady state: C(h) || B(h+1) || A(h+2), one strip of each per step.
        for s in range(NT):
            phase_a_strip(0, s)
        skgs = skg_prefetch(0)
        for s in range(NT):
            phase_b_strip(0, s, skgs)
            phase_a_strip(1, s)
        for h in range(H):
            ats, sats = xbar_prefetch(h)
            if h + 1 < H:
                skgs = skg_prefetch(h + 1)
            op = o_ps.tile([D, N], F32, tag="o", name="op")
            for i in range(NT):
                phase_c_mt(h, i, op, ats, sats)
                if h + 1 < H:
                    phase_b_strip(h + 1, i, skgs)
                if h + 2 < H:
                    phase_a_strip(h + 2, i)
            otile_evict(h, op)

        # ---- output projection ----
        for s in range(NT):
            fp = mm_ps.tile([P, C], F32, tag="mm", name="fp")
            for ct in range(CT):
                nc.tensor.matmul(
                    fp[:], oT_all[ct][:, s * P:(s + 1) * P], wo_sb[ct][:],
                    start=(ct == 0), stop=(ct == CT - 1))
            fo = work.tile([P, C], F32, tag="fo", name="fo")
            nc.vector.tensor_add(fo[:], fp[:], bias_sb[:])
            nc.sync.dma_start(out=out[s * P:(s + 1) * P, :], in_=fo[:])

    _split_multi_waits(nc)
    return nc


def kernel(x, Wq, Wk, Wv, Er_q, Er_k, Er_v, Wo, bo, **kw):
    if "nc" not in _CACHE:
        _CACHE["nc"] = _build()
    nc = _CACHE["nc"]
    bf = ml_dtypes.bfloat16
    x = np.asarray(x, dtype=np.float32)
    base = {
        "wq": np.asarray(Wq, np.float32).astype(bf),
        "wk": np.asarray(Wk, np.float32).astype(bf),
        "wv": np.asarray(Wv, np.float32).astype(bf),
        "wo": np.asarray(Wo, np.float32).astype(bf),
        "eq8T": np.ascontiguousarray(np.asarray(Er_q, np.float32).T * 8.0).astype(bf),
        "ekT": np.ascontiguousarray(np.asarray(Er_k, np.float32).T).astype(bf),
        "erv": np.asarray(Er_v, np.float32).astype(bf),
        "biasb": np.ascontiguousarray(
            np.broadcast_to(np.asarray(bo, np.float32), (P, C))),
        "ident": np.eye(P, dtype=ml_dtypes.float8_e4m3),
    }
    in_maps = []
    for b in range(x.shape[0]):
        m = dict(base)
        m["xT"] = np.ascontiguousarray(x[b].T).astype(bf)
        in_maps.append(m)
    _CACHE["in_maps"] = in_maps
    res = run_bass_kernel_spmd(nc, in_maps, list(range(len(in_maps))))
    return np.stack(
        [np.asarray(res.results[i]["out"], dtype=np.float32)
         for i in range(len(in_maps))])


# revision 21
# speedup vs baseline: 1.4463x; 1.0125x over previous
"""AxialAttention (relative-position skew attention) Trainium2 Bass kernel.

Sharding: data-parallel over batch B=8 across 8 NeuronCores (one batch
element per core, SPMD). Each core computes its full attention locally;
no collectives.

Per-core algorithm (N=1024, C=512, H=8, d=64):
  qT_h = 0.125 * (Wq_h^T @ x^T)        [64, N]  (scale folded into q)
  kT_h = Wk_h^T @ x^T                  [64, N]
  V    = x @ Wv                        [N, C]
  S    = q_h @ k_h^T                   (PE, bf16)
  G    = q_h @ (8*Er_q)^T + k_h @ Er_k^T
  skew(G) via DRAM zero-stuffed buffer trick:
     write G rows into Z viewed [N, N+1] cols 1..N (col 0 = zeros);
     the skewed matrix is exactly Z.flat[N : N+N*N] viewed [N, N].
  P = exp(S + skew(G)); rowsum via ACT accum_out; normalize (DVE)
  write P rows into another stuffed buffer Za;
  sv^T + VEr^T accumulated in one PSUM group:
     sv^T  = sum_mt  V[mt, h]^T-as-lhsT @ (P^T tile via DMA-xbar of Za plain view)
     VEr^T = sum_mt  Er_v[mt]-as-lhsT  @ (skew(P)^T tile via DMA-xbar of Za skew view)
  out = concat_heads(o)^T-as-lhsT @ Wo + bo
"""

import numpy as np
import ml_dtypes

import concourse.bass as bass
import concourse.mybir as mybir
import concourse.tile as tile
from concourse.bass_utils import run_bass_kernel_spmd
from contextlib import ExitStack

BF = mybir.dt.bfloat16
F8 = mybir.dt.float8e4
F32 = mybir.dt.float32
P = 128
N = 1024
C = 512
D = 64
H = 8
NT = N // P   # 8 strips/tiles
CT = C // P   # 4 c-tiles
CHW = 512     # matmul N-chunk (one PSUM bank of fp32)
CH = N // CHW # 2 chunks

_CACHE = {}


def _install_lane_partitioning():
    """Split HWDGE DMAs across both physical rings (qSPDynamicHW via
    nc.sync, qActDynamicHW via nc.scalar) for parallel transfer streams.
    Tile's DMA-completion sems assume per-lane FIFO completion, which
    breaks when two rings share sem lanes — so partition the 8 lanes by
    issuing engine: SP -> lanes 0-3, ACT -> lanes 4-7."""
    import concourse.tile_sem_assignment as tsa
    from concourse.tile_scheduler import DMAInst
    if getattr(tsa.TileClockTick, "_lane_partitioned", False):
        return
    orig = tsa.TileClockTick._assign_tick

    def patched(self, inst):
        if isinstance(inst, DMAInst) and inst.engine != mybir.EngineType.Pool:
            if inst.engine == mybir.EngineType.Activation:
                i = getattr(self, "_act_lane", 0)
                self.next_hw_dma_idx = 4 + (i % 4)
                self._act_lane = i + 1
            else:
                i = getattr(self, "_sp_lane", 0)
                self.next_hw_dma_idx = i % 4
                self._sp_lane = i + 1
        return orig(self, inst)

    tsa.TileClockTick._assign_tick = patched
    tsa.TileClockTick._lane_partitioned = True


def _split_multi_waits(nc):
    """Walrus in this toolchain allows only ONE sync-wait per instruction
    (setupSyncWait: 'Too many sync wait commands'). Tile emits multi-wait
    instructions (esp. the kernel-tail Drain). Split extras into preceding
    single-wait InstEventSemaphore ops on the same engine queue."""
    n = 0
    for bb in nc.m.functions[0].blocks:
        insts = list(bb.instructions)
        new = []
        for inst in insts:
            si = inst.sync_info
            waits = list(si.on_wait) if si is not None and si.on_wait else []
            if len(waits) > 1:
                for w in waits[:-1]:
                    n += 1
                    new.append(mybir.InstEventSemaphore(
                        name=f"I-wsplit-{n}",
                        engine=inst.engine, ins=[], outs=[],
                        sync_info=mybir.SyncInfo(on_wait=[w], on_update=[])))
                si.on_wait = waits[-1:]
            new.append(inst)
        bb.instructions = new
    return n


def _build():
    _install_lane_partitioning()
    nc = bass.Bass()
    xT = nc.declare_dram_parameter("xT", [C, N], BF, isOutput=False)
    wq = nc.declare_dram_parameter("wq", [C, C], BF, isOutput=False)
    wk = nc.declare_dram_parameter("wk", [C, C], BF, isOutput=False)
    wv = nc.declare_dram_parameter("wv", [C, C], BF, isOutput=False)
    wo = nc.declare_dram_parameter("wo", [C, C], BF, isOutput=False)
    eq8T = nc.declare_dram_parameter("eq8T", [D, N], BF, isOutput=False)
    ekT = nc.declare_dram_parameter("ekT", [D, N], BF, isOutput=False)
    erv = nc.declare_dram_parameter("erv", [N, D], BF, isOutput=False)
    biasb = nc.declare_dram_parameter("biasb", [P, C], F32, isOutput=False)
    ident = nc.declare_dram_parameter("ident", [P, P], F8, isOutput=False)
    out = nc.declare_dram_parameter("out", [N, C], F32, isOutput=True)

    Exp = mybir.ActivationFunctionType.Exp
    ZSZ = N * (N + 1)

    with ExitStack() as ctx:
        tc = ctx.enter_context(tile.TileContext(nc))
        const = ctx.enter_context(tc.tile_pool(name="const", bufs=1))
        work = ctx.enter_context(tc.tile_pool(name="work", bufs=3))
        gpool = ctx.enter_context(tc.tile_pool(name="gev", bufs=3))
        apool = ctx.enter_context(tc.tile_pool(name="attn", bufs=2))
        tpool = ctx.enter_context(tc.tile_pool(name="trp", bufs=1))
        mm_ps = ctx.enter_context(tc.tile_pool(name="mmps", bufs=3, space="PSUM"))
        o_ps = ctx.enter_context(tc.tile_pool(name="ops", bufs=1, space="PSUM"))
        dram = ctx.enter_context(tc.tile_pool(name="dram", bufs=1, space="DRAM"))

        # ---- load constants ----
        xT_sb = []
        for i in range(CT):
            t = const.tile([P, N], BF, name=f"xT{i}")
            nc.sync.dma_start(out=t[:], in_=xT[i * P:(i + 1) * P, :])
            xT_sb.append(t)

        def load_w(param, nm):
            ts = []
            for i in range(CT):
                t = const.tile([P, C], BF, name=f"{nm}{i}")
                nc.sync.dma_start(out=t[:], in_=param[i * P:(i + 1) * P, :])
                ts.append(t)
            return ts

        wq_sb = load_w(wq, "wq")
        wk_sb = load_w(wk, "wk")
        wv_sb = load_w(wv, "wv")
        wo_sb = load_w(wo, "wo")

        ee_sb = const.tile([P, N], BF, name="ee")
        nc.sync.dma_start(out=ee_sb[0:D, :], in_=eq8T[:, :])
        nc.sync.dma_start(out=ee_sb[D:2 * D, :], in_=ekT[:, :])
        erv_sb = []
        for mt in range(NT):
            t = const.tile([P, D], BF, name=f"erv{mt}")
            nc.sync.dma_start(out=t[:], in_=erv[mt * P:(mt + 1) * P, :])
            erv_sb.append(t)
        bias_sb = const.tile([P, C], F32, name="bias")
        nc.sync.dma_start(out=bias_sb[:], in_=biasb[:, :])
        id_sb = const.tile([P, P], F8, name="ident")
        nc.sync.dma_start(out=id_sb[:], in_=ident[:, :])
        # ---- DRAM stuffed scratch buffers (ping-pong across heads) ----
        # The stuffed zero column is written as part of each row-strip
        # write (tiles carry a zeroed col 0), so writes stay contiguous.
        zgs, zas = [], []
        for i in range(2):
            zgs.append(dram.tile([ZSZ], F8, name=f"zg{i}"))
            zas.append(dram.tile([ZSZ], BF, name=f"za{i}"))

        # ---- projections ----
        # qkT_h: [q_h^T (rows 0:64) ; k_h^T (rows 64:128)] for the stacked
        # G matmul (K=128). kT_h: k_h^T at partition base 0 for S's rhs.
        qkT_h = [const.tile([P, N], BF, name=f"qkT{h}") for h in range(H)]
        kT_h = [const.tile([D, N], BF, name=f"kT{h}") for h in range(H)]

        def project(w_sb, scale, place):
            for ct in range(CT):
                pair = work.tile([P, N], BF, tag="pair", name=f"pair{ct}")
                for ch in range(CH):
                    ps = mm_ps.tile([P, CHW], F32, tag="mm", name="psp")
                    for kt in range(CT):
                        nc.tensor.matmul(
                            ps[:],
                            w_sb[kt][:, ct * P:(ct + 1) * P],
                            xT_sb[kt][:, ch * CHW:(ch + 1) * CHW],
                            start=(kt == 0),
                            stop=(kt == CT - 1),
                        )
                    nc.scalar.mul(pair[:, ch * CHW:(ch + 1) * CHW], ps[:], scale)
                place(ct, pair)

        def place_q(ct, pair):
            nc.sync.dma_start(out=qkT_h[2 * ct][0:D, :], in_=pair[0:D, :])
            nc.sync.dma_start(out=qkT_h[2 * ct + 1][0:D, :], in_=pair[D:2 * D, :])

        def place_k(ct, pair):
            nc.sync.dma_start(out=qkT_h[2 * ct][D:2 * D, :], in_=pair[0:D, :])
            nc.sync.dma_start(out=qkT_h[2 * ct + 1][D:2 * D, :], in_=pair[D:2 * D, :])
            nc.sync.dma_start(out=kT_h[2 * ct][0:D, :], in_=pair[0:D, :])
            nc.sync.dma_start(out=kT_h[2 * ct + 1][0:D, :], in_=pair[D:2 * D, :])

        project(wq_sb, 0.125, place_q)
        project(wk_sb, 1.0, place_k)

        vfull = []
        for mt in range(NT):
            ps = mm_ps.tile([P, C], F32, tag="mm", name="psv")
            for kt in range(CT):
                nc.tensor.matmul(
                    ps[:],
                    xT_sb[kt][:, mt * P:(mt + 1) * P],
                    wv_sb[kt][:],
                    start=(kt == 0),
                    stop=(kt == CT - 1),
                )
            t = const.tile([P, C], BF, name=f"v{mt}")
            nc.scalar.copy(t[:], ps[:])
            vfull.append(t)

        oT_all = [const.tile([P, N], BF, name=f"oT{ct}") for ct in range(CT)]

        # ---- per-head attention (software-pipelined) ----
        # A(h): G strips -> DRAM; B(h): S + skew-add + softmax -> DRAM;
        # C(h): xbar-transposed reads + sv/VEr matmuls. C(h) is interleaved
        # with A(h+1) so PE always has independent matmuls to chew on while
        # the xbar reads stream in. Plain DMAs ride SWDGE (gpsimd) to keep
        # the SP HWDGE ring exclusively for the xbar transposes.
        ROWB = P * (N + 1)

        def views(h):
            zg, za = zgs[h % 2], zas[h % 2]
            return (zg, za,
                    za[:].rearrange("(n m) -> n m", m=N + 1),
                    zg[N:N + N * N].rearrange("(n m) -> n m", m=N),
                    za[N:N + N * N].rearrange("(n m) -> n m", m=N))

        def phase_a_strip(h, s):
            zg = zgs[h % 2]
            gp = mm_ps.tile([P, N], F32, tag="mm", name="gp")
            for ch in range(CH):
                sl = slice(ch * CHW, (ch + 1) * CHW)
                nc.tensor.matmul(
                    gp[:, sl], qkT_h[h][:, s * P:(s + 1) * P], ee_sb[:, sl],
                    start=True, stop=True, skip_group_check=True)
            gsb = gpool.tile([P, N + 1], F8, tag="gsb", name="gsb")
            nc.vector.memset(gsb[:, 0:1], 0.0)
            nc.vector.tensor_copy(gsb[:, 1:N + 1], gp[:])
            dst = zg[s * ROWB:(s + 1) * ROWB].rearrange("(p m) -> p m", m=N + 1)
            nc.sync.dma_start(out=dst[:, :], in_=gsb[:])

        def skg_prefetch(h):
            _, _, _, zg_skew, _ = views(h)
            skgs = []
            for s in range(NT):
                skg = apool.tile([P, N], F8, tag=f"skg{s}", bufs=1,
                                 name=f"skg{s}")
                nc.sync.dma_start(out=skg[:], in_=zg_skew[s * P:(s + 1) * P, :])
                skgs.append(skg)
            return skgs

        def phase_b_strip(h, s, skgs):
            za = zas[h % 2]
            sp = mm_ps.tile([P, N], F32, tag="mm", name="sp")
            for ch in range(CH):
                sl = slice(ch * CHW, (ch + 1) * CHW)
                nc.tensor.matmul(
                    sp[:, sl], qkT_h[h][0:D, s * P:(s + 1) * P],
                    kT_h[h][0:D, sl],
                    start=True, stop=False, skip_group_check=True)
                nc.tensor.matmul(
                    sp[:, sl], id_sb[:], skgs[s][:, sl],
                    start=False, stop=True, skip_group_check=True)
            aexp = apool.tile([P, N], BF, tag="aexp", name="aexp")
            rs = work.tile([P, 1], F32, tag="rs", name="rs")
            nc.scalar.activation(aexp[:], sp[:], Exp, accum_out=rs[:])
            rinv = work.tile([P, 1], F32, tag="rinv", name="rinv")
            nc.vector.reciprocal(rinv[:], rs[:])
            anorm = apool.tile([P, N + 1], BF, tag="anorm", name="anorm")
            nc.vector.memset(anorm[:, 0:1], 0.0)
            nc.vector.tensor_scalar_mul(anorm[:, 1:N + 1], aexp[:], rinv[:])
            dst = za[s * ROWB:(s + 1) * ROWB].rearrange("(p m) -> p m", m=N + 1)
            nc.sync.dma_start(out=dst[:, :], in_=anorm[:])

        def xbar_prefetch(h):
            _, _, za_rows, _, za_skew = views(h)
            ats, sats = [], []
            for mt in range(NT):
                at = tpool.tile([P, N], BF, tag=f"at{mt}", bufs=1,
                                name=f"at{mt}")
                nc.sync.dma_start(
                    out=at[:], in_=za_rows[:, 1 + mt * P:1 + (mt + 1) * P],
                    transpose=True)
                sat = tpool.tile([P, N], BF, tag=f"sat{mt}", bufs=1,
                                 name=f"sat{mt}")
                nc.sync.dma_start(
                    out=sat[:], in_=za_skew[:, mt * P:(mt + 1) * P],
                    transpose=True)
                ats.append(at)
                sats.append(sat)
            return ats, sats

        def phase_c_mt(h, mt, op, ats, sats):
            for ch in range(CH):
                sl = slice(ch * CHW, (ch + 1) * CHW)
                nc.tensor.matmul(
                    op[:, sl], vfull[mt][:, h * D:(h + 1) * D],
                    ats[mt][:, sl],
                    start=(mt == 0), stop=False, skip_group_check=True)
                nc.tensor.matmul(
                    op[:, sl], erv_sb[mt][:], sats[mt][:, sl],
                    start=False, stop=(mt == NT - 1),
                    skip_group_check=True)
            # keep the PE HAM activity monitor warm through the xbar window
            nc.tensor.ldweights(id_sb[:, :])

        def otile_evict(h, op):
            ot = work.tile([D, N], BF, tag="ot", name="ot")
            nc.scalar.copy(ot[:], op[:])
            nc.sync.dma_start(
                out=oT_all[h // 2][(h % 2) * D:(h % 2 + 1) * D, :], in_=ot[:])

        # Steady state: C(h) || B(h+1) || A(h+2), one strip of each per step.
        for s in range(NT):
            phase_a_strip(0, s)
        skgs = skg_prefetch(0)
        for s in range(NT):
            phase_b_strip(0, s, skgs)
            phase_a_strip(1, s)
        for h in range(H):
            ats, sats = xbar_prefetch(h)
            if h + 1 < H:
                skgs = skg_prefetch(h + 1)
            op = o_ps.tile([D, N], F32, tag="o", name="op")
            for i in range(NT):
                phase_c_mt(h, i, op, ats, sats)
                if h + 1 < H:
                    phase_b_strip(h + 1, i, skgs)
                if h + 2 < H:
                    phase_a_strip(h + 2, i)
            otile_evict(h, op)

        # ---- output projection ----
        for s in range(NT):
            fp = mm_ps.tile([P, C], F32, tag="mm", name="fp")
            for ct in range(CT):
                nc.tensor.matmul(
                    fp[:], oT_all[ct][:, s * P:(s + 1) * P], wo_sb[ct][:],
                    start=(ct == 0), stop=(ct == CT - 1))
            fo = work.tile([P, C], F32, tag="fo", name="fo")
            nc.vector.tensor_add(fo[:], fp[:], bias_sb[:])
            nc.sync.dma_start(out=out[s * P:(s + 1) * P, :], in_=fo[:])

    _split_multi_waits(nc)
    return nc


def kernel(x, Wq, Wk, Wv, Er_q, Er_k, Er_v, Wo, bo, **kw):
    if "nc" not in _CACHE:
        _CACHE["nc"] = _build()
    nc = _CACHE["nc"]
    bf = ml_dtypes.bfloat16
    x = np.asarray(x, dtype=np.float32)
    base = {
        "wq": np.asarray(Wq, np.float32).astype(bf),
        "wk": np.asarray(Wk, np.float32).astype(bf),
        "wv": np.asarray(Wv, np.float32).astype(bf),
        "wo": np.asarray(Wo, np.float32).astype(bf),
        "eq8T": np.ascontiguousarray(np.asarray(Er_q, np.float32).T * 8.0).astype(bf),
        "ekT": np.ascontiguousarray(np.asarray(Er_k, np.float32).T).astype(bf),
        "erv": np.asarray(Er_v, np.float32).astype(bf),
        "biasb": np.ascontiguousarray(
            np.broadcast_to(np.asarray(bo, np.float32), (P, C))),
        "ident": np.eye(P, dtype=ml_dtypes.float8_e4m3),
    }
    in_maps = []
    for b in range(x.shape[0]):
        m = dict(base)
        m["xT"] = np.ascontiguousarray(x[b].T).astype(bf)
        in_maps.append(m)
    _CACHE["in_maps"] = in_maps
    res = run_bass_kernel_spmd(nc, in_maps, list(range(len(in_maps))))
    return np.stack(
        [np.asarray(res.results[i]["out"], dtype=np.float32)
         for i in range(len(in_maps))])
